# revision 15
# baseline (speedup 1.0000x reference)
"""LocallyConnected2d (3x3, stride 1, pad 1) Trainium2 kernel, 8-way spatial-parallel.

out[n,o,h,w] = sum_{c,i,k} weight[o,h,w,c,i,k] * xpad[n,c,h+i,w+k] + bias[o,h,w]

Sharding: output rows h are split 7-per-core across 8 NeuronCores. Each core
streams its private 1/8 weight slice exactly once, in bf16 (host-cast); the
kernel is DMA-bound end to end. Measured on this HW: 128-partition DMAs
sustain ~400 GB/s while 96-partition ones cap at ~220 GB/s, so everything is
shaped to 128 partitions:

- Output rows are processed in PAIRS (2p, 2p+1). The contraction for a pair
  spans 4 padded input rows x 32 channels = 128 partitions. Weights for the
  pair interleave as [slot, g, o] per column with 25% structural zeros
  (g=0 uses partition rows 0..95, g=1 uses 32..127) — 32% more bytes, but
  at 1.8x the bandwidth it's a clear net win. The odd row 6 runs alone with
  zero-padded weight partitions 96..127.
- The matmul for padded column j (0..57): lhsT = x column [128, n=32]
  (stationary), rhs = pair weights [128, window <= 3 slots x 2g x 32o]
  (moving), accumulated in PSUM over the 3 columns j = w..w+2 feeding each
  pixel w. start=True on HW zeroes the WHOLE PSUM bank then writes the
  addressed columns (verified empirically), so only the group's first matmul
  sets it.
- Pair p's PSUM/output live at partitions 32p..32p+31 (matmul tile_position),
  so all four row-blocks evict into one [128, x] staging tile and leave as a
  single fast 128-partition output DMA.
- bias is a post-GEMM additive constant; the host adds it after gathering
  (the spec's bias is all-zeros anyway).

All input DMAs are issued up front into persistent SBUF tiles (per-row DMAs
that depend on compute were measured to stall ~6us each behind the weight
packet backlog in the DMA engine rings). The host upcasts the bf16 output.
"""

import numpy as np
import ml_dtypes

import concourse.bass as bass
import concourse.mybir as mybir
import concourse.tile as tile
from concourse.vector_clock import ScopedClock, VectorClock
from concourse.bass_utils import run_bass_kernel_spmd

N, C, H, W = 32, 32, 56, 56
O = 32
NCORES = 8
R = H // NCORES          # output rows per core: 3 pairs + row 6
JW = W + 2               # padded input columns
GP = 8                   # pixels per PSUM group, pairs (8*2*32 = 512 fp32/bank)
GP6 = 14                 # pixels per PSUM group, row 6 (14*32 = 448)
KP = 3 * C

BF16 = ml_dtypes.bfloat16

_patched = False


def _patch_tile_drain():
    """The walrus build in this container rejects >1 sem wait on an InstDrain.
    Move the Tile tail-drain's waits onto one sync-engine nop per processor
    (same-engine in-order issue makes this equivalent), leaving the drain bare.
    """
    global _patched
    if _patched:
        return

    def _drain_and_barrier(self, tick_clock, wait_clock):
        gc = tick_clock.global_clock
        n = len(gc)
        for proc in range(n):
            t = gc[proc]
            if t <= 0:
                continue
            vec = [0] * n
            vec[proc] = t
            nop = self.nc.sync.nop(nofuse=True)
            wait_clock.add_sem_waits(nop.ins, ScopedClock({None: VectorClock(vec)}))
        self.nc.sync.drain()
        self.nc.all_engine_barrier()
        assert self.sems is not None
        popped = self.nc._tile_sem_poison_stack.pop()
        assert popped is self._sem_poison
        self.nc.clear_and_free_semaphores(list(self.sems.allocated().values()))
        self.nc.all_engine_barrier()

    tile.TileContext._drain_and_barrier = _drain_and_barrier
    _patched = True


def _split_multi_waits(nc):
    """This container's walrus accepts at most one semaphore wait per lowered
    instruction (matmul waits land on its single-slot LDWEIGHTS). Hoist all
    but the last wait of every instruction onto same-engine NoOps just before
    it; same-engine in-order issue preserves the wait semantics."""
    ctr = 0
    for fn in nc.m.functions:
        for bb in fn.blocks:
            out = []
            for inst in bb.instructions:
                si = inst.sync_info
                if si is not None and len(si.on_wait) > 1:
                    waits = list(si.on_wait)
                    for w in waits[:-1]:
                        ctr += 1
                        nop = mybir.InstNoOp(
                            name=f"{inst.name}-wsplit-{ctr}",
                            sync_info=mybir.SyncInfo(on_wait=[w], on_update=[]),
                            bass_nofuse=True,
                            engine=inst.engine,
                        )
                        out.append(nop)
                    si.on_wait = [waits[-1]]
                out.append(inst)
            bb.instructions = out
    return ctr


_nc_cache = None


def _build_nc():
    global _nc_cache
    if _nc_cache is not None:
        return _nc_cache
    _patch_tile_drain()
    nc = bass.Bass()
    f32 = mybir.dt.float32
    bf16 = mybir.dt.bfloat16
    # weights are split into column chunks of ~2 PSUM groups each (with a
    # 2-column overlap at chunk seams) so the first matmuls start after ~1MB
    # of stream instead of a full 2.85MB pair slab.
    wp = nc.dram_tensor("wp", [3, 128, 64 * 6 * O], bf16, kind="ExternalInput")
    w6 = nc.dram_tensor("w6", [128, 60 * 3 * O], bf16, kind="ExternalInput")
    xh = nc.dram_tensor("xh", [R + 3, C, JW, N], bf16, kind="ExternalInput")
    out = nc.dram_tensor("out", [96, 2 * O * W], bf16, kind="ExternalOutput")
    ou6 = nc.dram_tensor("ou6", [32, O * W], bf16, kind="ExternalOutput")

    with tile.TileContext(nc) as tc:
        with (
            tc.tile_pool(name="singles", bufs=1) as singles,
            tc.tile_pool(name="ps", bufs=8, space="PSUM") as pspool,
        ):
            PCH = [(0, 18), (16, 18), (32, 18), (48, 10)]
            CH6 = [(0, 30), (28, 30)]
            wpt = [
                [
                    singles.tile([128, ln * 6 * O], bf16, name=f"wp{p}c{c}")
                    for c, (_, ln) in enumerate(PCH)
                ]
                for p in range(3)
            ]
            w6t = [
                singles.tile([128, ln * 3 * O], bf16, name=f"w6c{c}")
                for c, (_, ln) in enumerate(CH6)
            ]
            x_t = [
                singles.tile([128, JW * N], bf16, name=f"x{p}") for p in range(4)
            ]
            o_m = singles.tile([96, 2 * O * W], bf16, name="om")
            o_6 = singles.tile([32, O * W], bf16, name="o6")

            # All input DMAs up front, every one a fast 128-partition shape:
            # x on the scalar queue, weights on the sync queue.
            for p in range(3):
                nc.scalar.dma_start(
                    out=x_t[p],
                    in_=xh[2 * p : 2 * p + 4].rearrange("r c j n -> (r c) (j n)"),
                )
                off = 0
                for c, (_, ln) in enumerate(PCH):
                    nc.sync.dma_start(
                        out=wpt[p][c],
                        in_=wp[p][:, off * 6 * O : (off + ln) * 6 * O],
                    )
                    off += ln
            nc.scalar.dma_start(
                out=x_t[3], in_=xh[6:10].rearrange("r c j n -> (r c) (j n)")
            )
            off = 0
            for c, (_, ln) in enumerate(CH6):
                nc.sync.dma_start(
                    out=w6t[c], in_=w6[:, off * 3 * O : (off + ln) * 3 * O]
                )
                off += ln

            for p in range(3):
                lo_p, hi_p = 32 * p, 32 * p + 32
                for g in range(W // GP):
                    wa = g * GP
                    last_j = wa + GP + 1
                    ch = g // 2
                    j0 = PCH[ch][0]
                    wct = wpt[p][ch]
                    ps = pspool.tile([128, 2 * GP * O], f32)
                    for j in range(wa, wa + GP + 2):
                        lo = max(j - 2, wa)
                        hi = min(j, wa + GP - 1)
                        slo = lo - j + 2
                        nwin = hi - lo + 1
                        jl = j - j0
                        nc.tensor.matmul(
                            ps[lo_p:hi_p, (lo - wa) * 64 : (hi + 1 - wa) * 64],
                            lhsT=x_t[p][:, j * N : (j + 1) * N],
                            rhs=wct[
                                :, jl * 192 + slo * 64 : jl * 192 + (slo + nwin) * 64
                            ],
                            start=(j == wa),
                            stop=(j == last_j),
                        )
                    # evict psum [n, (w', g, o)] into o_m [n@32p, (g, o, w)]
                    src = ps[lo_p:hi_p].rearrange("p (w g o) -> p g o w", g=2, o=O)
                    dst = o_m[lo_p:hi_p].rearrange("p (g o w) -> p g o w", g=2, o=O)[
                        :, :, :, wa : wa + GP
                    ]
                    if g % 2 == 0:
                        nc.vector.tensor_copy(out=dst, in_=src)
                    else:
                        nc.scalar.copy(out=dst, in_=src)

            for g in range(W // GP6):
                wa = g * GP6
                last_j = wa + GP6 + 1
                ch = g // 2
                j0 = CH6[ch][0]
                wct = w6t[ch]
                ps = pspool.tile([128, 2 * GP * O], f32)
                for j in range(wa, wa + GP6 + 2):
                    lo = max(j - 2, wa)
                    hi = min(j, wa + GP6 - 1)
                    slo = lo - j + 2
                    nwin = hi - lo + 1
                    jl = j - j0
                    nc.tensor.matmul(
                        ps[0:32, (lo - wa) * O : (hi + 1 - wa) * O],
                        lhsT=x_t[3][:, j * N : (j + 1) * N],
                        rhs=wct[:, jl * 96 + slo * O : jl * 96 + (slo + nwin) * O],
                        start=(j == wa),
                        stop=(j == last_j),
                    )
                src = ps[0:32, : GP6 * O].rearrange("p (w o) -> p o w", o=O)
                dst = o_6.rearrange("p (o w) -> p o w", o=O)[:, :, wa : wa + GP6]
                if g % 2 == 0:
                    nc.vector.tensor_copy(out=dst, in_=src)
                else:
                    nc.scalar.copy(out=dst, in_=src)

            nc.scalar.dma_start(out=out[:], in_=o_m)
            nc.scalar.dma_start(out=ou6[:], in_=o_6)
    _split_multi_waits(nc)
    _nc_cache = nc
    return nc


def _pack_core(weight, xp, core):
    h0 = core * R
    Wc = weight[:, h0 : h0 + R]  # [O, R, W, C, 3, 3]
    # per-row packing: w96[i, c, r, j, slot, o] = weight[o, h0+r, j-2+slot, c, i, 2-slot]
    w96 = np.zeros((3, C, R, JW, 3, O), np.float32)
    for wp_ in range(3):
        k = 2 - wp_
        src = Wc[:, :, :, :, :, k]  # [O, R, W, C, I]
        w96[:, :, :, 2 - wp_ : 2 - wp_ + W, wp_, :] = src.transpose(4, 3, 1, 2, 0)
    # pairs: [p, (qr c), j, (slot g o)]; g=0 on partition rows 0..95, g=1 on 32..127
    wpk = np.zeros((3, 4, C, JW, 3, 2, O), np.float32)
    for p in range(3):
        wpk[p, 0:3, :, :, :, 0, :] = w96[:, :, 2 * p]
        wpk[p, 1:4, :, :, :, 1, :] = w96[:, :, 2 * p + 1]
    wpk = wpk.reshape(3, 128, JW, 6 * O)
    PCH = [(0, 18), (16, 18), (32, 18), (48, 10)]
    CH6 = [(0, 30), (28, 30)]
    wpk = np.concatenate([wpk[:, :, j0 : j0 + ln] for j0, ln in PCH], axis=2)
    wpk = wpk.reshape(3, 128, 64 * 6 * O)
    w6f = np.zeros((128, JW, 3 * O), np.float32)
    w6f[:KP] = w96[:, :, 6].reshape(KP, JW, 3 * O)
    w6k = np.concatenate([w6f[:, j0 : j0 + ln] for j0, ln in CH6], axis=1)
    w6k = w6k.reshape(128, 60 * 3 * O)
    # x: local padded rows 0..8 plus one zero row so row-6's DMA is 128-partition
    xhc = np.zeros((R + 3, C, JW, N), np.float32)
    xhc[: R + 2] = xp[:, :, h0 : h0 + R + 2, :].transpose(2, 1, 3, 0)
    return {
        "wp": wpk.astype(BF16),
        "w6": w6k.astype(BF16),
        "xh": np.ascontiguousarray(xhc).astype(BF16),
    }


def kernel(x, weight, bias, _want_trace=False):
    x = np.asarray(x, dtype=np.float32)
    weight = np.asarray(weight, dtype=np.float32)
    bias = np.asarray(bias, dtype=np.float32)
    nc = _build_nc()
    xp = np.pad(x, ((0, 0), (0, 0), (1, 1), (1, 1)))
    in_maps = [_pack_core(weight, xp, c) for c in range(NCORES)]
    res = run_bass_kernel_spmd(
        nc, in_maps, core_ids=list(range(NCORES)), trace=_want_trace
    )
    full = np.empty((N, O, H, W), np.float32)
    for c in range(NCORES):
        arr = res.results[c]["out"].astype(np.float32)  # [96, 2*O*W]
        h0 = c * R
        for p in range(3):
            slab = arr[32 * p : 32 * p + 32].reshape(N, 2, O, W)
            full[:, :, h0 + 2 * p, :] = slab[:, 0]
            full[:, :, h0 + 2 * p + 1, :] = slab[:, 1]
        a6 = res.results[c]["ou6"].astype(np.float32)
        full[:, :, h0 + 6, :] = a6.reshape(N, O, W)
    full += bias[0]  # bias is a post-GEMM constant; free on the host
    if _want_trace:
        return full, res
    return full
